# revision 6
# baseline (speedup 1.0000x reference)
"""Trainium2 Bass kernel for nn_AttentionBlock_68624987455817.

Pre-LN causal self-attention block + MLP (B=8, L=1024, E=768, H=12, D=64).

Sharding: data-parallel over batch B=8 across the 8 NeuronCores (one batch
element per core, weights replicated, no collectives). Each core runs the
full block on its [1024, 768] slice.

v2 restructure vs the original: fully pipelined per-tile LayerNorms (stats
finalized per token tile, no all-tile barrier), PE-array transposes for both
dtypes (the DMA-transpose path stalled the PE ~40us per LN), x1 kept
SBUF-resident (the DRAM scratch roundtrip cost ~50us of PE idle), software
pipelining of transpose vs consumer matmuls in the PE queue, and part of the
SELU elementwise work moved to the otherwise-idle GpSimd engine.

Per-core dataflow (activations kept feature-major through the matmuls):
  phB    LN1 per x tile -> z tile -> PE transpose -> z1T [E, L]; v = z1 @ wv
         interleaved one tile behind (ones column per head makes the P@V
         matmul emit softmax row-sums)
  phC    per head pair: qk chunks (q pre-scaled 1/sqrt(D)), then
         S^T = k_h^T q_h -> exp -> P^T (masked); [O^T; sums] = Vaug^T P^T;
         normalize via fast reciprocal + gpsimd partition broadcast.
  phD    x1 = O @ wproj + x (SBUF-resident); LN2 stats+apply per tile;
         z2 PE transposes interleaved one tile behind the proj matmuls
  phE    hT = selu(wfc^T @ z2T)      (wfc pre-scaled by selu lambda)
  phF    out = h @ wout + x1         (token-major, two column passes)

Matmul operand dtype is selectable (KERNEL_MM_DT env): "bf16" (default;
rel err ~3e-3) or "f32r" (rel err ~2e-4). Both stream at the same PE rate
on this hardware; bf16 halves SBUF/DMA. Accumulation is always fp32.
Softmax skips the max-subtraction (|S| <= ~8 for LN'd inputs so exp cannot
overflow in fp32); causal masking zeroes P^T diagonal blocks.

LN scales fold into the following weight matrices host-side; LN biases and
all linear biases fold into per-feature biases that are only materialized
on-chip when nonzero (all zero for this problem's inputs).
"""
import os
import sys
from contextlib import ExitStack

sys.path.insert(0, "/opt/trn_rl_repo")

import numpy as np
import ml_dtypes

import concourse.bass as bass
from concourse import bacc
import concourse.mybir as mybir
from concourse.tile import TileContext
from concourse import bass_utils
from concourse.masks import make_identity

F32 = mybir.dt.float32
F32R = mybir.dt.float32r
BF16 = mybir.dt.bfloat16
AF = mybir.ActivationFunctionType
OP = mybir.AluOpType
AX = mybir.AxisListType

P = 128
L = 1024
E = 768
H = 12
D = 64
DA = D + 1           # V columns + ones column (row-sum trick)
EC = E // P          # 6 feature chunks
LT = L // P          # 8 token tiles
QC = L // 512        # 2 query chunks
KC2 = 4 * E // P     # 24 chunks of the MLP hidden dim
NCORES = 8

SELU_LAMBDA = 1.0507009873554805
SELU_ALPHA = 1.6732632423543772
SELU_LA = SELU_LAMBDA * SELU_ALPHA
LN_EPS = 1e-6

_last_results = None
_build_cache = {}


class _Ctx:
    pass


def _persist_setup(g, gates, MDT):
    nc, pers = g.nc, g.pers
    # mask_tri[p, f] = 1.0 if f >= p else 0.0 (keep where k <= q).
    # Built in f32 (f32r memset/affine_select fail walrus codegen).
    mask_f32 = pers.tile([P, P], F32)
    nc.gpsimd.memset(mask_f32[:], 0.0)
    nc.gpsimd.affine_select(
        out=mask_f32[:], in_=mask_f32[:],
        compare_op=OP.is_ge, fill=1.0, base=-1,
        pattern=[[-1, P]], channel_multiplier=1,
    )
    if MDT == F32R:
        g.mask_tri = mask_f32[:].bitcast(F32R)
    else:
        mask_b = pers.tile([P, P], BF16)
        nc.vector.tensor_copy(mask_b[:], mask_f32[:])
        g.mask_tri = mask_b[:]
    g.ones_f32 = pers.tile([P, LT * H], F32)
    nc.vector.memset(g.ones_f32[:], 1.0)
    g.eps_b = pers.tile([P, 1], F32)
    nc.vector.memset(g.eps_b[:], LN_EPS)
    g.lnla_b = pers.tile([P, 1], F32)
    nc.vector.memset(g.lnla_b[:], float(np.log(SELU_LA)))

    g.m1 = pers.tile([P, LT], F32)
    g.sq1 = pers.tile([P, LT], F32)
    g.r1 = pers.tile([P, LT], F32)
    g.tmp8 = pers.tile([P, LT], F32)
    g.m2 = pers.tile([P, LT], F32)
    g.sq2 = pers.tile([P, LT], F32)
    g.r2 = pers.tile([P, LT], F32)

    g.bqk_sb = g.bv_sb = g.bproj_sb = g.bfce_sb = g.bfcl_sb = g.bout_sb = None
    if gates["bqk"]:
        g.bqk_sb = pers.tile([P, 2 * EC], F32)
        nc.sync.dma_start(g.bqk_sb[:], g.bqk_d.rearrange("(c p) -> p c", p=P))
    if gates["bv"]:
        g.bv_sb = pers.tile([P, E], F32)
        nc.sync.dma_start(g.bv_sb[:], g.bv_d.to_broadcast((P, E)))
    if gates["bproj"]:
        g.bproj_sb = pers.tile([P, E], F32)
        nc.sync.dma_start(g.bproj_sb[:], g.bproj_d.to_broadcast((P, E)))
    if gates["bfc"]:
        g.bfce_sb = pers.tile([P, KC2], F32)
        nc.sync.dma_start(g.bfce_sb[:], g.bfce_d.rearrange("(c p) -> p c", p=P))
        g.bfcl_sb = pers.tile([P, KC2], F32)
        nc.sync.dma_start(g.bfcl_sb[:], g.bfcl_d.rearrange("(c p) -> p c", p=P))
    if gates["bout"]:
        g.bout_sb = pers.tile([P, E], F32)
        nc.sync.dma_start(g.bout_sb[:], g.bout_d.to_broadcast((P, E)))

    ident = pers.tile([P, P], F32)
    make_identity(nc, ident)
    g.ident_m = pers.tile([P, P], MDT)
    nc.vector.tensor_copy(g.ident_m[:], ident[:])


def _ln_tile(g, xt, msum, sqv, rv, t):
    """Per-tile LN stats + finalize: msum/sqv/rv are [P, LT] column t."""
    nc = g.nc
    nc.vector.tensor_reduce(msum[:, t:t + 1], xt[:], AX.X, OP.add)
    sqs = g.ln_scr.tile([P, E], F32, tag="sq")
    nc.scalar.activation(sqs[:], xt[:], AF.Square, accum_out=sqv[:, t:t + 1])
    nc.vector.tensor_scalar_mul(msum[:, t:t + 1], msum[:, t:t + 1], 1.0 / E)
    nc.vector.tensor_tensor(g.tmp8[:, t:t + 1], msum[:, t:t + 1],
                            msum[:, t:t + 1], OP.mult)
    nc.vector.tensor_scalar_mul(sqv[:, t:t + 1], sqv[:, t:t + 1], 1.0 / E)
    nc.vector.tensor_tensor(sqv[:, t:t + 1], sqv[:, t:t + 1],
                            g.tmp8[:, t:t + 1], OP.subtract)
    nc.scalar.activation(sqv[:, t:t + 1], sqv[:, t:t + 1], AF.Sqrt,
                         bias=g.eps_b[:])
    nc.vector.reciprocal(rv[:, t:t + 1], sqv[:, t:t + 1])


def _phB(g, gates, MDT, tc):
    """LN1 -> z1T (PE transposes) with v = z1 @ wv pipelined one tile behind."""
    nc = g.nc
    with (
        tc.tile_pool(name="zp", bufs=3) as zp,
        tc.tile_pool(name="wvp", bufs=1) as wvp,
        tc.tile_pool(name="pstr", bufs=4, space="PSUM") as pstr,
        tc.tile_pool(name="psv", bufs=2, space="PSUM") as psv,
    ):
        wv_sb = wvp.tile([P, EC, E], MDT)
        nc.sync.dma_start(wv_sb[:], g.wvv[:])
        nc.vector.tensor_copy(
            g.v_aug[:, :, :, D:DA],
            g.ones_f32[:].rearrange("p (t h o) -> p t h o", h=H, o=1))

        g.xtiles = []
        for t in range(LT):
            xt = g.xp.tile([P, E], F32, tag="x")
            nc.sync.dma_start(xt[:], g.xv[:, t, :])
            g.xtiles.append(xt)

        def make_v(t):
            def emit():
                for (c0, cw) in ((0, 512), (512, 256)):
                    pv = psv.tile([P, 512], F32, tag="vmm")
                    for kc in range(EC):
                        nc.tensor.matmul(
                            pv[:, :cw], g.z1T[:, kc, t * P:(t + 1) * P],
                            wv_sb[:, kc, c0:c0 + cw],
                            start=(kc == 0), stop=(kc == EC - 1),
                        )
                    h0 = c0 // D
                    nh = cw // D
                    dst = g.v_aug[:, t, h0:h0 + nh, 0:D]
                    if gates["bv"]:
                        nc.vector.tensor_tensor(
                            dst, pv[:, :cw].rearrange("p (h d) -> p h d", d=D),
                            g.bv_sb[:, c0:c0 + cw].rearrange(
                                "p (h d) -> p h d", d=D),
                            OP.add)
                    else:
                        nc.any.tensor_copy(
                            out=dst,
                            in_=pv[:, :cw].rearrange("p (h d) -> p h d", d=D))
            return emit

        v_emit = None
        for t in range(LT):
            _ln_tile(g, g.xtiles[t], g.m1, g.sq1, g.r1, t)
            zt = zp.tile([P, E], MDT, tag="z")
            nc.vector.tensor_scalar(
                zt[:], g.xtiles[t][:], g.m1[:, t:t + 1], g.r1[:, t:t + 1],
                OP.subtract, OP.mult,
            )
            for c in range(EC):
                pt = pstr.tile([P, P], MDT, tag="tr")
                nc.tensor.transpose(pt[:], zt[:, c * P:(c + 1) * P],
                                    g.ident_m[:])
                nc.any.tensor_copy(out=g.z1T[:, c, t * P:(t + 1) * P],
                                   in_=pt[:])
            if v_emit is not None:
                v_emit()
            v_emit = make_v(t)
        v_emit()


def _phC_qk(g, gates, c, qkpp, wqs, psqk):
    """qk matmuls for head pair c: oc=c (q), oc=EC+c (k)."""
    nc = g.nc
    qk_pair = qkpp.tile([P, 2, L], g.MDT, tag="qkpair")
    for i, oc in enumerate((c, EC + c)):
        wt = wqs.tile([P, EC, P], g.MDT, tag="wqk")
        nc.sync.dma_start(wt[:], g.wqkv[:, :, oc * P:(oc + 1) * P])
        psums = [psqk.tile([P, 512], F32, tag="mm", name=f"qkps{lc}")
                 for lc in range(QC)]
        for kc in range(EC):
            for lc in range(QC):
                nc.tensor.matmul(
                    psums[lc][:], wt[:, kc, :],
                    g.z1T[:, kc, lc * 512:(lc + 1) * 512],
                    start=(kc == 0), stop=(kc == EC - 1),
                )
        for lc in range(QC):
            dst = qk_pair[:, i, lc * 512:(lc + 1) * 512]
            if gates["bqk"]:
                nc.scalar.activation(dst, psums[lc][:], AF.Identity,
                                     bias=g.bqk_sb[:, oc:oc + 1])
            else:
                nc.any.tensor_copy(out=dst, in_=psums[lc][:])
    return qk_pair


def _phC_attn(g, c, qc, qk_pair, PT, recp, ps3s, ps3v):
    """S^T -> exp/mask -> P@V + normalize for head pair c, query chunk qc."""
    nc = g.nc
    q0 = qc * 512
    for kt in range(4 * qc, 4 * (qc + 1)):
        s0 = kt * P
        segs = [(s0, 512), (512, L)] if s0 < 512 else [(s0, L)]
        psss = []
        for par in range(2):
            rows = slice(par * D, par * D + D)
            pss = ps3s.tile([P, L], F32, tag="st", name=f"pss{par}")
            lhs = qk_pair[rows, 1, s0:s0 + P]
            for (a, b) in segs:
                nc.tensor.matmul(pss[:, a:b], lhs, qk_pair[rows, 0, a:b],
                                 start=True, stop=True)
            psss.append(pss)
        for par in range(2):
            pt_buf = PT[par]
            nc.scalar.activation(pt_buf[:, kt, s0:L], psss[par][:, s0:L],
                                 AF.Exp)
            nc.vector.tensor_tensor(
                pt_buf[:, kt, s0:s0 + P], pt_buf[:, kt, s0:s0 + P],
                g.mask_tri, OP.mult,
            )
    # P@V for both heads: lhsT = [V_h | 1] so psum row 64 carries the
    # softmax row-sums; the reciprocal (computed on one row, SBUF — the
    # custom DVE op reads garbage from PSUM) is partition-broadcast on the
    # idle GpSimd.
    for par in range(2):
        h = 2 * c + par
        pt_buf = PT[par]
        pso = ps3v.tile([P, 512], F32, tag="pv")
        kts = list(range(4 * (qc + 1)))
        for j, kt in enumerate(kts):
            a = max(kt * P, q0)
            nc.tensor.matmul(pso[0:DA, a - q0:512], g.v_aug[:, kt, h, :],
                             pt_buf[:, kt, a:q0 + 512],
                             start=(j == 0), stop=(j == len(kts) - 1))
        o_rows = slice(par * D, par * D + D)
        srow = recp.tile([P, 512], F32, tag="sr")
        nc.vector.tensor_copy(srow[0:1, :], pso[D:DA, :])
        rec = recp.tile([P, 512], F32, tag="rc")
        nc.vector.reciprocal_approx_fast(rec[0:1, :], srow[0:1, :])
        recb = recp.tile([P, 512], F32, tag="rb")
        nc.gpsimd.partition_broadcast(recb[0:D, :], rec[0:1, :])
        nc.vector.tensor_tensor(
            OT_slice(g, o_rows, c, q0), pso[0:D, :], recb[0:D, :], OP.mult,
        )


def OT_slice(g, o_rows, c, q0):
    return g.OT[o_rows, c, q0:q0 + 512]


def _phC(g, gates, MDT, tc):
    nc = g.nc
    with (
        tc.tile_pool(name="qkpp", bufs=2) as qkpp,
        tc.tile_pool(name="wqks", bufs=2) as wqs,
        tc.tile_pool(name="ptp", bufs=1) as ptp,
        tc.tile_pool(name="recp", bufs=2) as recp,
        tc.tile_pool(name="psqk", bufs=2, space="PSUM") as psqk,
        tc.tile_pool(name="ps3s", bufs=2, space="PSUM") as ps3s,
        tc.tile_pool(name="ps3v", bufs=2, space="PSUM") as ps3v,
    ):
        PT = [ptp.tile([P, LT, L], MDT, tag=f"pt{i}", name=f"pt{i}")
              for i in range(2)]
        # Prefetch the next pair's qk matmuls ahead of this pair's attention
        # in the PE queue: the qk matmuls have no ACT dependencies, so the PE
        # never stalls on the softmax exp chain.
        qk_next = _phC_qk(g, gates, 0, qkpp, wqs, psqk)
        for c in range(EC):  # head pair (2c, 2c+1)
            qk_pair = qk_next
            if c + 1 < EC:
                qk_next = _phC_qk(g, gates, c + 1, qkpp, wqs, psqk)
            for qc in range(QC):
                _phC_attn(g, c, qc, qk_pair, PT, recp, ps3s, ps3v)


def _phD(g, gates, MDT, tc):
    """proj + residual (x1 SBUF-resident) + per-tile LN2 -> z2T."""
    nc = g.nc
    for oq in range(4):
        nc.sync.dma_start(g.wo_sb[:, 6 * oq:6 * (oq + 1), :],
                          g.woutv[:, 6 * oq:6 * (oq + 1), :])
    with (
        tc.tile_pool(name="wpp", bufs=1) as wpp,
        tc.tile_pool(name="z2p", bufs=3) as z2p,
        tc.tile_pool(name="psp4", bufs=3, space="PSUM") as ps4,
        tc.tile_pool(name="pstr2", bufs=4, space="PSUM") as pstr2,
    ):
        wproj_sb = wpp.tile([P, EC, E], MDT)
        nc.sync.dma_start(wproj_sb[:], g.wprojv[:])

        def make_tr2(t, z2t):
            def emit():
                for c in range(EC):
                    pt = pstr2.tile([P, P], MDT, tag="tr2")
                    nc.tensor.transpose(pt[:], z2t[:, c * P:(c + 1) * P],
                                        g.ident_m[:])
                    nc.any.tensor_copy(out=g.z2T[:, c, t * P:(t + 1) * P],
                                       in_=pt[:])
            return emit

        g.x1tiles = []
        tr_emit = None
        for t in range(LT):
            xt = g.xtiles[t]
            x1t = g.x1p.tile([P, E], F32, tag="x1")
            for (c0, cw) in ((0, 512), (512, 256)):
                pt = ps4.tile([P, 512], F32, tag="mm")
                for kc in range(EC):
                    nc.tensor.matmul(
                        pt[:, :cw], g.OT[:, kc, t * P:(t + 1) * P],
                        wproj_sb[:, kc, c0:c0 + cw],
                        start=(kc == 0), stop=(kc == EC - 1),
                    )
                dst = x1t[:, c0:c0 + cw]
                if gates["bproj"]:
                    nc.vector.tensor_tensor(dst, pt[:, :cw],
                                            g.bproj_sb[:, c0:c0 + cw], OP.add)
                    nc.vector.tensor_tensor(dst, dst, xt[:, c0:c0 + cw],
                                            OP.add)
                else:
                    nc.vector.tensor_tensor(dst, pt[:, :cw],
                                            xt[:, c0:c0 + cw], OP.add)
            g.x1tiles.append(x1t)
            _ln_tile(g, x1t, g.m2, g.sq2, g.r2, t)
            z2t = z2p.tile([P, E], MDT, tag="z2")
            nc.vector.tensor_scalar(
                z2t[:], x1t[:], g.m2[:, t:t + 1], g.r2[:, t:t + 1],
                OP.subtract, OP.mult,
            )
            if tr_emit is not None:
                tr_emit()
            tr_emit = make_tr2(t, z2t)
        tr_emit()


def _phE(g, gates, MDT, tc):
    """fc + selu -> hT (feature-major)."""
    nc = g.nc
    with (
        tc.tile_pool(name="wfcs", bufs=3) as wfs,
        tc.tile_pool(name="selu", bufs=2) as slp,
        tc.tile_pool(name="ps5", bufs=4, space="PSUM") as ps5,
    ):
        for oc in range(KC2):
            wt = wfs.tile([P, EC, P], MDT, tag="wfc")
            nc.sync.dma_start(wt[:], g.wfcv[:, :, oc * P:(oc + 1) * P])
            for lc in range(QC):
                pt = ps5.tile([P, 512], F32, tag="mm")
                for kc in range(EC):
                    nc.tensor.matmul(
                        pt[:], wt[:, kc, :],
                        g.z2T[:, kc, lc * 512:(lc + 1) * 512],
                        start=(kc == 0), stop=(kc == EC - 1),
                    )
                pe_t = slp.tile([P, 512], F32, tag="pe")
                bias = g.bfce_sb[:, oc:oc + 1] if gates["bfc"] else g.lnla_b[:]
                nc.scalar.activation(pe_t[:], pt[:], AF.Exp, bias=bias,
                                     scale=1.0 / SELU_LAMBDA)
                a_t = slp.tile([P, 512], F32, tag="at")
                nc.gpsimd.tensor_scalar(a_t[:], pe_t[:], SELU_LA, SELU_LA,
                                        OP.min, OP.subtract)
                dst = g.hT[:, oc, lc * 512:(lc + 1) * 512]
                if gates["bfc"]:
                    rl = slp.tile([P, 512], F32, tag="rl")
                    nc.vector.tensor_scalar(rl[:], pt[:],
                                            g.bfcl_sb[:, oc:oc + 1],
                                            0.0, OP.add, OP.max)
                    nc.vector.tensor_tensor(dst, rl[:], a_t[:], OP.add)
                else:
                    nc.vector.scalar_tensor_tensor(dst, pt[:], 0.0, a_t[:],
                                                   OP.max, OP.add)


def _phF(g, gates, MDT, tc):
    """out = h @ wout + x1, two column passes, straight to DRAM."""
    nc = g.nc
    with (
        tc.tile_pool(name="osA", bufs=3) as osp,
        tc.tile_pool(name="ps6", bufs=4, space="PSUM") as ps6,
    ):
        for (c0, cw) in ((0, 512), (512, 256)):
            for t in range(LT):
                pt = ps6.tile([P, 512], F32, tag="mm")
                for kc in range(KC2):
                    nc.tensor.matmul(
                        pt[:, :cw], g.hT[:, kc, t * P:(t + 1) * P],
                        g.wo_sb[:, kc, c0:c0 + cw],
                        start=(kc == 0), stop=(kc == KC2 - 1),
                    )
                ot = osp.tile([P, 512], F32, tag="ot")
                if gates["bout"]:
                    nc.vector.tensor_tensor(ot[:, :cw], pt[:, :cw],
                                            g.bout_sb[:, c0:c0 + cw], OP.add)
                    nc.vector.tensor_tensor(ot[:, :cw], ot[:, :cw],
                                            g.x1tiles[t][:, c0:c0 + cw],
                                            OP.add)
                else:
                    nc.vector.tensor_tensor(ot[:, :cw], pt[:, :cw],
                                            g.x1tiles[t][:, c0:c0 + cw],
                                            OP.add)
                nc.sync.dma_start(g.outv[:, t, c0:c0 + cw], ot[:, :cw])


def _build(gates, mm_dt_name):
    MDT = {"f32r": F32R, "bf16": BF16}[mm_dt_name]

    nc = bacc.Bacc("TRN2", target_bir_lowering=False)
    g = _Ctx()
    g.nc = nc
    g.MDT = MDT

    x_d = nc.dram_tensor("x", [L, E], F32, kind="ExternalInput")
    wqk_d = nc.dram_tensor("wqk", [E, 2 * E], MDT, kind="ExternalInput")
    wv_d = nc.dram_tensor("wv", [E, E], MDT, kind="ExternalInput")
    wproj_d = nc.dram_tensor("wproj", [E, E], MDT, kind="ExternalInput")
    wfc_d = nc.dram_tensor("wfc", [E, 4 * E], MDT, kind="ExternalInput")
    wout_d = nc.dram_tensor("wout", [4 * E, E], MDT, kind="ExternalInput")
    out_d = nc.dram_tensor("out", [L, E], F32, kind="ExternalOutput")

    if gates["bqk"]:
        g.bqk_d = nc.dram_tensor("bqk", [2 * E], F32, kind="ExternalInput")
    if gates["bv"]:
        g.bv_d = nc.dram_tensor("bv", [E], F32, kind="ExternalInput")
    if gates["bproj"]:
        g.bproj_d = nc.dram_tensor("bproj", [E], F32, kind="ExternalInput")
    if gates["bfc"]:
        g.bfce_d = nc.dram_tensor("bfce", [4 * E], F32, kind="ExternalInput")
        g.bfcl_d = nc.dram_tensor("bfcl", [4 * E], F32, kind="ExternalInput")
    if gates["bout"]:
        g.bout_d = nc.dram_tensor("bout", [E], F32, kind="ExternalInput")

    g.xv = x_d.rearrange("(t p) e -> p t e", p=P)            # [128, 8, 768]
    g.wqkv = wqk_d.rearrange("(c p) m -> p c m", p=P)        # [128, 6, 1536]
    g.wvv = wv_d.rearrange("(c p) m -> p c m", p=P)          # [128, 6, 768]
    g.wprojv = wproj_d.rearrange("(c p) m -> p c m", p=P)    # [128, 6, 768]
    g.wfcv = wfc_d.rearrange("(c p) m -> p c m", p=P)        # [128, 6, 3072]
    g.woutv = wout_d.rearrange("(c p) m -> p c m", p=P)      # [128, 24, 768]
    g.outv = out_d.rearrange("(t p) e -> p t e", p=P)

    with TileContext(nc) as tc, ExitStack() as es:
        g.pers = es.enter_context(tc.tile_pool(name="persist", bufs=1))
        _persist_setup(g, gates, MDT)

        g.bigp = es.enter_context(tc.tile_pool(name="big", bufs=1))
        g.x1p = es.enter_context(tc.tile_pool(name="x1p", bufs=LT))
        g.ln_scr = es.enter_context(tc.tile_pool(name="lnscr", bufs=2))
        g.wop = es.enter_context(tc.tile_pool(name="wop", bufs=1))

        g.z1T = g.bigp.tile([P, EC, L], MDT, tag="zT", name="z1T")
        g.OT = g.bigp.tile([P, EC, L], MDT, tag="ot", name="OT")
        g.wo_sb = g.wop.tile([P, KC2, E], MDT, name="wo")

        with (
            tc.tile_pool(name="xp", bufs=LT) as xp,
            tc.tile_pool(name="vaugp", bufs=1) as vaugp,
        ):
            g.xp = xp
            g.v_aug = vaugp.tile([P, LT, H, DA], MDT)

            _phB(g, gates, MDT, tc)
            _phC(g, gates, MDT, tc)

            g.z2T = g.bigp.tile([P, EC, L], MDT, tag="zT", name="z2T")
            _phD(g, gates, MDT, tc)

        with tc.tile_pool(name="htp", bufs=1) as htp:
            g.hT = htp.tile([P, KC2, L], MDT, name="hT")
            _phE(g, gates, MDT, tc)
            _phF(g, gates, MDT, tc)

    nc.finalize()
    return nc


def kernel(**inputs):
    global _last_results

    mm_dt_name = os.environ.get("KERNEL_MM_DT", "bf16")

    def arr(name):
        return np.ascontiguousarray(np.asarray(inputs[name], dtype=np.float32))

    x = arr("x")                       # [8, 1024, 768]
    g1 = arr("ln1_scale")
    b1 = arr("ln1_bias")
    w_qkv = arr("w_qkv")               # [768, 2304]
    b_qkv = arr("b_qkv")
    w_proj = arr("w_proj")
    b_proj = arr("b_proj")
    g2 = arr("ln2_scale")
    b2 = arr("ln2_bias")
    w_fc = arr("w_fc")
    b_fc = arr("b_fc")
    w_out = arr("w_out")
    b_out = arr("b_out")

    qscale = np.float32(1.0 / np.sqrt(D))

    w3 = w_qkv.reshape(E, H, 3, D)
    qw = (w3[:, :, 0, :].reshape(E, E) * qscale)
    kw = w3[:, :, 1, :].reshape(E, E)
    vw = w3[:, :, 2, :].reshape(E, E)
    wqk = np.ascontiguousarray(
        np.concatenate([qw, kw], axis=1) * g1[:, None]).astype(np.float32)
    wv = np.ascontiguousarray(vw * g1[:, None]).astype(np.float32)

    bq3 = (b1 @ w_qkv + b_qkv).reshape(H, 3, D)
    bqk = np.concatenate(
        [bq3[:, 0, :].reshape(E) * qscale, bq3[:, 1, :].reshape(E)]).astype(np.float32)
    bv = np.ascontiguousarray(bq3[:, 2, :].reshape(E)).astype(np.float32)

    wfc_p = np.ascontiguousarray(
        w_fc * g2[:, None] * np.float32(SELU_LAMBDA)).astype(np.float32)
    bfc_eff = (b2 @ w_fc + b_fc).astype(np.float32)
    bfce = (bfc_eff + np.float32(np.log(SELU_LA))).astype(np.float32)
    bfcl = (bfc_eff * np.float32(SELU_LAMBDA)).astype(np.float32)

    gates = {
        "bqk": bool(np.any(bqk != 0)),
        "bv": bool(np.any(bv != 0)),
        "bproj": bool(np.any(b_proj != 0)),
        "bfc": bool(np.any(bfc_eff != 0)),
        "bout": bool(np.any(b_out != 0)),
    }

    key = (tuple(sorted(gates.items())), mm_dt_name)
    if key not in _build_cache:
        _build_cache[key] = _build(gates, mm_dt_name)
    nc = _build_cache[key]

    wdt = np.float32 if mm_dt_name == "f32r" else ml_dtypes.bfloat16

    def wcast(a):
        return np.ascontiguousarray(a.astype(wdt))

    base = {
        "wqk": wcast(wqk), "wv": wcast(wv),
        "wproj": wcast(w_proj),
        "wfc": wcast(wfc_p),
        "wout": wcast(w_out),
    }
    if gates["bqk"]:
        base["bqk"] = bqk
    if gates["bv"]:
        base["bv"] = bv
    if gates["bproj"]:
        base["bproj"] = np.ascontiguousarray(b_proj)
    if gates["bfc"]:
        base["bfce"] = bfce
        base["bfcl"] = bfcl
    if gates["bout"]:
        base["bout"] = np.ascontiguousarray(b_out)

    in_maps = [dict(base, x=np.ascontiguousarray(x[c])) for c in range(NCORES)]
    res = bass_utils.run_bass_kernel_spmd(nc, in_maps, core_ids=list(range(NCORES)))
    _last_results = res
    out = np.stack([res.results[c]["out"] for c in range(NCORES)], axis=0)
    return out.astype(np.float32)


# revision 7
# speedup vs baseline: 1.8200x; 1.8200x over previous
"""Trainium2 Bass kernel for nn_AttentionBlock_68624987455817.

Pre-LN causal self-attention block + MLP (B=8, L=1024, E=768, H=12, D=64).

Sharding: data-parallel over batch B=8 across the 8 NeuronCores (one batch
element per core, weights replicated, no collectives). Each core runs the
full block on its [1024, 768] slice.

v2 restructure vs the original: fully pipelined per-tile LayerNorms (stats
finalized per token tile, no all-tile barrier), PE-array transposes for both
dtypes (the DMA-transpose path stalled the PE ~40us per LN), x1 kept
SBUF-resident (the DRAM scratch roundtrip cost ~50us of PE idle), software
pipelining of transpose vs consumer matmuls in the PE queue, and part of the
SELU elementwise work moved to the otherwise-idle GpSimd engine.

Per-core dataflow (activations kept feature-major through the matmuls):
  phB    LN1 per x tile -> z tile -> PE transpose -> z1T [E, L]; v = z1 @ wv
         interleaved one tile behind (ones column per head makes the P@V
         matmul emit softmax row-sums)
  phC    per head pair: qk chunks (q pre-scaled 1/sqrt(D)), then
         S^T = k_h^T q_h -> exp -> P^T (masked); [O^T; sums] = Vaug^T P^T;
         normalize via fast reciprocal + gpsimd partition broadcast.
  phD    x1 = O @ wproj + x (SBUF-resident); LN2 stats+apply per tile;
         z2 PE transposes interleaved one tile behind the proj matmuls
  phE    hT = selu(wfc^T @ z2T)      (wfc pre-scaled by selu lambda)
  phF    out = h @ wout + x1         (token-major, two column passes)

Matmul operand dtype is selectable (KERNEL_MM_DT env): "bf16" (default;
rel err ~3e-3) or "f32r" (rel err ~2e-4). Both stream at the same PE rate
on this hardware; bf16 halves SBUF/DMA. Accumulation is always fp32.
Softmax skips the max-subtraction (|S| <= ~8 for LN'd inputs so exp cannot
overflow in fp32); causal masking zeroes P^T diagonal blocks.

LN scales fold into the following weight matrices host-side; LN biases and
all linear biases fold into per-feature biases that are only materialized
on-chip when nonzero (all zero for this problem's inputs).
"""
import os
import sys
from contextlib import ExitStack

sys.path.insert(0, "/opt/trn_rl_repo")

import numpy as np
import ml_dtypes

import concourse.bass as bass
from concourse import bacc
import concourse.mybir as mybir
from concourse.tile import TileContext
from concourse import bass_utils
from concourse.masks import make_identity

F32 = mybir.dt.float32
F32R = mybir.dt.float32r
BF16 = mybir.dt.bfloat16
AF = mybir.ActivationFunctionType
OP = mybir.AluOpType
AX = mybir.AxisListType

P = 128
L = 1024
E = 768
H = 12
D = 64
DA = D + 1           # V columns + ones column (row-sum trick)
EC = E // P          # 6 feature chunks
LT = L // P          # 8 token tiles
QC = L // 512        # 2 query chunks
KC2 = 4 * E // P     # 24 chunks of the MLP hidden dim
NCORES = 8

SELU_LAMBDA = 1.0507009873554805
SELU_ALPHA = 1.6732632423543772
SELU_LA = SELU_LAMBDA * SELU_ALPHA
LN_EPS = 1e-6

_last_results = None
_build_cache = {}


class _Ctx:
    pass


def _persist_setup(g, gates, MDT):
    nc, pers = g.nc, g.pers
    # mask_tri[p, f] = 1.0 if f >= p else 0.0 (keep where k <= q).
    # Built in f32 (f32r memset/affine_select fail walrus codegen).
    mask_f32 = pers.tile([P, P], F32)
    nc.gpsimd.memset(mask_f32[:], 0.0)
    nc.gpsimd.affine_select(
        out=mask_f32[:], in_=mask_f32[:],
        compare_op=OP.is_ge, fill=1.0, base=-1,
        pattern=[[-1, P]], channel_multiplier=1,
    )
    if MDT == F32R:
        g.mask_tri = mask_f32[:].bitcast(F32R)
    else:
        mask_b = pers.tile([P, P], BF16)
        nc.vector.tensor_copy(mask_b[:], mask_f32[:])
        g.mask_tri = mask_b[:]
    g.ones_f32 = pers.tile([P, LT * H], F32)
    nc.vector.memset(g.ones_f32[:], 1.0)
    g.eps_b = pers.tile([P, 1], F32)
    nc.vector.memset(g.eps_b[:], LN_EPS)
    g.lnla_b = pers.tile([P, 1], F32)
    nc.vector.memset(g.lnla_b[:], float(np.log(SELU_LA)))

    g.m1 = pers.tile([P, LT], F32)
    g.sq1 = pers.tile([P, LT], F32)
    g.r1 = pers.tile([P, LT], F32)
    g.tmp8 = pers.tile([P, LT], F32)
    g.m2 = pers.tile([P, LT], F32)
    g.sq2 = pers.tile([P, LT], F32)
    g.r2 = pers.tile([P, LT], F32)

    g.bqk_sb = g.bv_sb = g.bproj_sb = g.bfce_sb = g.bfcl_sb = g.bout_sb = None
    if gates["bqk"]:
        g.bqk_sb = pers.tile([P, 2 * EC], F32)
        nc.sync.dma_start(g.bqk_sb[:], g.bqk_d.rearrange("(c p) -> p c", p=P))
    if gates["bv"]:
        g.bv_sb = pers.tile([P, E], F32)
        nc.sync.dma_start(g.bv_sb[:], g.bv_d.to_broadcast((P, E)))
    if gates["bproj"]:
        g.bproj_sb = pers.tile([P, E], F32)
        nc.sync.dma_start(g.bproj_sb[:], g.bproj_d.to_broadcast((P, E)))
    if gates["bfc"]:
        g.bfce_sb = pers.tile([P, KC2], F32)
        nc.sync.dma_start(g.bfce_sb[:], g.bfce_d.rearrange("(c p) -> p c", p=P))
        g.bfcl_sb = pers.tile([P, KC2], F32)
        nc.sync.dma_start(g.bfcl_sb[:], g.bfcl_d.rearrange("(c p) -> p c", p=P))
    if gates["bout"]:
        g.bout_sb = pers.tile([P, E], F32)
        nc.sync.dma_start(g.bout_sb[:], g.bout_d.to_broadcast((P, E)))

    ident = pers.tile([P, P], F32)
    make_identity(nc, ident)
    g.ident_m = pers.tile([P, P], MDT)
    nc.vector.tensor_copy(g.ident_m[:], ident[:])


def _ln_tile(g, xt, msum, sqv, rv, t):
    """Per-tile LN stats + finalize: msum/sqv/rv are [P, LT] column t."""
    nc = g.nc
    nc.vector.tensor_reduce(msum[:, t:t + 1], xt[:], AX.X, OP.add)
    sqs = g.ln_scr.tile([P, E], F32, tag="sq")
    nc.scalar.activation(sqs[:], xt[:], AF.Square, accum_out=sqv[:, t:t + 1])
    nc.vector.tensor_scalar_mul(msum[:, t:t + 1], msum[:, t:t + 1], 1.0 / E)
    nc.vector.tensor_tensor(g.tmp8[:, t:t + 1], msum[:, t:t + 1],
                            msum[:, t:t + 1], OP.mult)
    nc.vector.tensor_scalar_mul(sqv[:, t:t + 1], sqv[:, t:t + 1], 1.0 / E)
    nc.vector.tensor_tensor(sqv[:, t:t + 1], sqv[:, t:t + 1],
                            g.tmp8[:, t:t + 1], OP.subtract)
    nc.scalar.activation(sqv[:, t:t + 1], sqv[:, t:t + 1], AF.Sqrt,
                         bias=g.eps_b[:])
    nc.vector.reciprocal(rv[:, t:t + 1], sqv[:, t:t + 1])


def _phB(g, gates, MDT, tc):
    """LN1 -> z1T (PE transposes) with v = z1 @ wv pipelined one tile behind."""
    nc = g.nc
    with (
        tc.tile_pool(name="zp", bufs=3) as zp,
        tc.tile_pool(name="wvp", bufs=1) as wvp,
        tc.tile_pool(name="pstr", bufs=4, space="PSUM") as pstr,
        tc.tile_pool(name="psv", bufs=2, space="PSUM") as psv,
    ):
        wv_sb = wvp.tile([P, EC, E], MDT)
        nc.sync.dma_start(wv_sb[:], g.wvv[:])
        nc.vector.tensor_copy(
            g.v_aug[:, :, :, D:DA],
            g.ones_f32[:].rearrange("p (t h o) -> p t h o", h=H, o=1))

        g.xtiles = []
        for t in range(LT):
            xt = g.xp.tile([P, E], F32, tag="x")
            nc.sync.dma_start(xt[:], g.xv[:, t, :])
            g.xtiles.append(xt)

        def make_v(t):
            def emit():
                for (c0, cw) in ((0, 512), (512, 256)):
                    pv = psv.tile([P, 512], F32, tag="vmm")
                    for kc in range(EC):
                        nc.tensor.matmul(
                            pv[:, :cw], g.z1T[:, kc, t * P:(t + 1) * P],
                            wv_sb[:, kc, c0:c0 + cw],
                            start=(kc == 0), stop=(kc == EC - 1),
                        )
                    h0 = c0 // D
                    nh = cw // D
                    dst = g.v_aug[:, t, h0:h0 + nh, 0:D]
                    if gates["bv"]:
                        nc.vector.tensor_tensor(
                            dst, pv[:, :cw].rearrange("p (h d) -> p h d", d=D),
                            g.bv_sb[:, c0:c0 + cw].rearrange(
                                "p (h d) -> p h d", d=D),
                            OP.add)
                    else:
                        nc.any.tensor_copy(
                            out=dst,
                            in_=pv[:, :cw].rearrange("p (h d) -> p h d", d=D))
            return emit

        v_emit = None
        for t in range(LT):
            _ln_tile(g, g.xtiles[t], g.m1, g.sq1, g.r1, t)
            zt = zp.tile([P, E], MDT, tag="z")
            nc.vector.tensor_scalar(
                zt[:], g.xtiles[t][:], g.m1[:, t:t + 1], g.r1[:, t:t + 1],
                OP.subtract, OP.mult,
            )
            for c in range(EC):
                pt = pstr.tile([P, P], MDT, tag="tr")
                nc.tensor.transpose(pt[:], zt[:, c * P:(c + 1) * P],
                                    g.ident_m[:])
                nc.any.tensor_copy(out=g.z1T[:, c, t * P:(t + 1) * P],
                                   in_=pt[:])
            if v_emit is not None:
                v_emit()
            v_emit = make_v(t)
        v_emit()


def _phC_qk(g, gates, c, qkpp, wqs, psqk):
    """qk matmuls for head pair c: oc=c (q), oc=EC+c (k)."""
    nc = g.nc
    qk_pair = qkpp.tile([P, 2, L], g.MDT, tag="qkpair")
    for i, oc in enumerate((c, EC + c)):
        wt = wqs.tile([P, EC, P], g.MDT, tag="wqk")
        nc.sync.dma_start(wt[:], g.wqkv[:, :, oc * P:(oc + 1) * P])
        psums = [psqk.tile([P, 512], F32, tag="mm", name=f"qkps{lc}")
                 for lc in range(QC)]
        for kc in range(EC):
            for lc in range(QC):
                nc.tensor.matmul(
                    psums[lc][:], wt[:, kc, :],
                    g.z1T[:, kc, lc * 512:(lc + 1) * 512],
                    start=(kc == 0), stop=(kc == EC - 1),
                )
        for lc in range(QC):
            dst = qk_pair[:, i, lc * 512:(lc + 1) * 512]
            if gates["bqk"]:
                nc.scalar.activation(dst, psums[lc][:], AF.Identity,
                                     bias=g.bqk_sb[:, oc:oc + 1])
            else:
                nc.any.tensor_copy(out=dst, in_=psums[lc][:])
    return qk_pair


def _phC_attn(g, c, qc, qk_pair, PT, recp, ps3s, ps3v):
    """S^T -> exp/mask -> P@V + normalize for head pair c, query chunk qc."""
    nc = g.nc
    q0 = qc * 512
    for kt in range(4 * qc, 4 * (qc + 1)):
        s0 = kt * P
        segs = [(s0, 512), (512, L)] if s0 < 512 else [(s0, L)]
        psss = []
        for par in range(2):
            rows = slice(par * D, par * D + D)
            pss = ps3s.tile([P, L], F32, tag="st", name=f"pss{par}")
            lhs = qk_pair[rows, 1, s0:s0 + P]
            for (a, b) in segs:
                nc.tensor.matmul(pss[:, a:b], lhs, qk_pair[rows, 0, a:b],
                                 start=True, stop=True)
            psss.append(pss)
        for par in range(2):
            pt_buf = PT[par]
            nc.scalar.activation(pt_buf[:, kt, s0:L], psss[par][:, s0:L],
                                 AF.Exp)
            nc.vector.tensor_tensor(
                pt_buf[:, kt, s0:s0 + P], pt_buf[:, kt, s0:s0 + P],
                g.mask_tri, OP.mult,
            )
    # P@V for both heads: lhsT = [V_h | 1] so psum row 64 carries the
    # softmax row-sums; the reciprocal (computed on one row, SBUF — the
    # custom DVE op reads garbage from PSUM) is partition-broadcast on the
    # idle GpSimd.
    for par in range(2):
        h = 2 * c + par
        pt_buf = PT[par]
        pso = ps3v.tile([P, 512], F32, tag="pv")
        kts = list(range(4 * (qc + 1)))
        for j, kt in enumerate(kts):
            a = max(kt * P, q0)
            nc.tensor.matmul(pso[0:DA, a - q0:512], g.v_aug[:, kt, h, :],
                             pt_buf[:, kt, a:q0 + 512],
                             start=(j == 0), stop=(j == len(kts) - 1))
        o_rows = slice(par * D, par * D + D)
        srow = recp.tile([P, 512], F32, tag="sr")
        nc.vector.tensor_copy(srow[0:1, :], pso[D:DA, :])
        rec = recp.tile([P, 512], F32, tag="rc")
        nc.vector.reciprocal_approx_fast(rec[0:1, :], srow[0:1, :])
        recb = recp.tile([P, 512], F32, tag="rb")
        nc.gpsimd.partition_broadcast(recb[0:D, :], rec[0:1, :])
        nc.vector.tensor_tensor(
            OT_slice(g, o_rows, c, q0), pso[0:D, :], recb[0:D, :], OP.mult,
        )


def OT_slice(g, o_rows, c, q0):
    return g.OT[o_rows, c, q0:q0 + 512]


def _phC(g, gates, MDT, tc):
    nc = g.nc
    with (
        tc.tile_pool(name="qkpp", bufs=2) as qkpp,
        tc.tile_pool(name="wqks", bufs=2) as wqs,
        tc.tile_pool(name="ptp", bufs=1) as ptp,
        tc.tile_pool(name="recp", bufs=2) as recp,
        tc.tile_pool(name="psqk", bufs=2, space="PSUM") as psqk,
        tc.tile_pool(name="ps3s", bufs=2, space="PSUM") as ps3s,
        tc.tile_pool(name="ps3v", bufs=2, space="PSUM") as ps3v,
    ):
        PT = [ptp.tile([P, LT, L], MDT, tag=f"pt{i}", name=f"pt{i}")
              for i in range(2)]
        # Prefetch the next pair's qk matmuls ahead of this pair's attention
        # in the PE queue: the qk matmuls have no ACT dependencies, so the PE
        # never stalls on the softmax exp chain.
        qk_next = _phC_qk(g, gates, 0, qkpp, wqs, psqk)
        for c in range(EC):  # head pair (2c, 2c+1)
            qk_pair = qk_next
            if c + 1 < EC:
                qk_next = _phC_qk(g, gates, c + 1, qkpp, wqs, psqk)
            for qc in range(QC):
                _phC_attn(g, c, qc, qk_pair, PT, recp, ps3s, ps3v)


def _phD(g, gates, MDT, tc):
    """proj + residual (x1 SBUF-resident) + per-tile LN2 -> z2T."""
    nc = g.nc
    for oq in range(4):
        nc.sync.dma_start(g.wo_sb[:, 6 * oq:6 * (oq + 1), :],
                          g.woutv[:, 6 * oq:6 * (oq + 1), :])
    with (
        tc.tile_pool(name="wpp", bufs=1) as wpp,
        tc.tile_pool(name="z2p", bufs=3) as z2p,
        tc.tile_pool(name="psp4", bufs=3, space="PSUM") as ps4,
        tc.tile_pool(name="pstr2", bufs=4, space="PSUM") as pstr2,
    ):
        wproj_sb = wpp.tile([P, EC, E], MDT)
        nc.sync.dma_start(wproj_sb[:], g.wprojv[:])

        def make_tr2(t, z2t):
            def emit():
                for c in range(EC):
                    pt = pstr2.tile([P, P], MDT, tag="tr2")
                    nc.tensor.transpose(pt[:], z2t[:, c * P:(c + 1) * P],
                                        g.ident_m[:])
                    nc.any.tensor_copy(out=g.z2T[:, c, t * P:(t + 1) * P],
                                       in_=pt[:])
            return emit

        g.x1tiles = []
        tr_emit = None
        for t in range(LT):
            xt = g.xtiles[t]
            x1t = g.x1p.tile([P, E], F32, tag="x1")
            for (c0, cw) in ((0, 512), (512, 256)):
                pt = ps4.tile([P, 512], F32, tag="mm")
                for kc in range(EC):
                    nc.tensor.matmul(
                        pt[:, :cw], g.OT[:, kc, t * P:(t + 1) * P],
                        wproj_sb[:, kc, c0:c0 + cw],
                        start=(kc == 0), stop=(kc == EC - 1),
                    )
                dst = x1t[:, c0:c0 + cw]
                if gates["bproj"]:
                    nc.vector.tensor_tensor(dst, pt[:, :cw],
                                            g.bproj_sb[:, c0:c0 + cw], OP.add)
                    nc.vector.tensor_tensor(dst, dst, xt[:, c0:c0 + cw],
                                            OP.add)
                else:
                    nc.vector.tensor_tensor(dst, pt[:, :cw],
                                            xt[:, c0:c0 + cw], OP.add)
            g.x1tiles.append(x1t)
            _ln_tile(g, x1t, g.m2, g.sq2, g.r2, t)
            z2t = z2p.tile([P, E], MDT, tag="z2")
            nc.vector.tensor_scalar(
                z2t[:], x1t[:], g.m2[:, t:t + 1], g.r2[:, t:t + 1],
                OP.subtract, OP.mult,
            )
            if tr_emit is not None:
                tr_emit()
            tr_emit = make_tr2(t, z2t)
        tr_emit()


def _phE(g, gates, MDT, tc):
    """fc + selu -> hT (feature-major)."""
    nc = g.nc
    with (
        tc.tile_pool(name="wfcs", bufs=3) as wfs,
        tc.tile_pool(name="selu", bufs=2) as slp,
        tc.tile_pool(name="ps5", bufs=4, space="PSUM") as ps5,
    ):
        for oc in range(KC2):
            wt = wfs.tile([P, EC, P], MDT, tag="wfc")
            nc.sync.dma_start(wt[:], g.wfcv[:, :, oc * P:(oc + 1) * P])
            for lc in range(QC):
                pt = ps5.tile([P, 512], F32, tag="mm")
                for kc in range(EC):
                    nc.tensor.matmul(
                        pt[:], wt[:, kc, :],
                        g.z2T[:, kc, lc * 512:(lc + 1) * 512],
                        start=(kc == 0), stop=(kc == EC - 1),
                    )
                pe_t = slp.tile([P, 512], F32, tag="pe")
                bias = g.bfce_sb[:, oc:oc + 1] if gates["bfc"] else g.lnla_b[:]
                nc.scalar.activation(pe_t[:], pt[:], AF.Exp, bias=bias,
                                     scale=1.0 / SELU_LAMBDA)
                a_t = slp.tile([P, 512], F32, tag="at")
                nc.vector.tensor_scalar(a_t[:], pe_t[:], SELU_LA, SELU_LA,
                                        OP.min, OP.subtract)
                dst = g.hT[:, oc, lc * 512:(lc + 1) * 512]
                if gates["bfc"]:
                    rl = slp.tile([P, 512], F32, tag="rl")
                    nc.vector.tensor_scalar(rl[:], pt[:],
                                            g.bfcl_sb[:, oc:oc + 1],
                                            0.0, OP.add, OP.max)
                    nc.vector.tensor_tensor(dst, rl[:], a_t[:], OP.add)
                else:
                    nc.vector.scalar_tensor_tensor(dst, pt[:], 0.0, a_t[:],
                                                   OP.max, OP.add)


def _phF(g, gates, MDT, tc):
    """out = h @ wout + x1, two column passes, straight to DRAM."""
    nc = g.nc
    with (
        tc.tile_pool(name="osA", bufs=3) as osp,
        tc.tile_pool(name="ps6", bufs=4, space="PSUM") as ps6,
    ):
        for (c0, cw) in ((0, 512), (512, 256)):
            for t in range(LT):
                pt = ps6.tile([P, 512], F32, tag="mm")
                for kc in range(KC2):
                    nc.tensor.matmul(
                        pt[:, :cw], g.hT[:, kc, t * P:(t + 1) * P],
                        g.wo_sb[:, kc, c0:c0 + cw],
                        start=(kc == 0), stop=(kc == KC2 - 1),
                    )
                ot = osp.tile([P, 512], F32, tag="ot")
                if gates["bout"]:
                    nc.vector.tensor_tensor(ot[:, :cw], pt[:, :cw],
                                            g.bout_sb[:, c0:c0 + cw], OP.add)
                    nc.vector.tensor_tensor(ot[:, :cw], ot[:, :cw],
                                            g.x1tiles[t][:, c0:c0 + cw],
                                            OP.add)
                else:
                    nc.vector.tensor_tensor(ot[:, :cw], pt[:, :cw],
                                            g.x1tiles[t][:, c0:c0 + cw],
                                            OP.add)
                nc.sync.dma_start(g.outv[:, t, c0:c0 + cw], ot[:, :cw])


def _build(gates, mm_dt_name):
    MDT = {"f32r": F32R, "bf16": BF16}[mm_dt_name]

    nc = bacc.Bacc("TRN2", target_bir_lowering=False)
    g = _Ctx()
    g.nc = nc
    g.MDT = MDT

    x_d = nc.dram_tensor("x", [L, E], F32, kind="ExternalInput")
    wqk_d = nc.dram_tensor("wqk", [E, 2 * E], MDT, kind="ExternalInput")
    wv_d = nc.dram_tensor("wv", [E, E], MDT, kind="ExternalInput")
    wproj_d = nc.dram_tensor("wproj", [E, E], MDT, kind="ExternalInput")
    wfc_d = nc.dram_tensor("wfc", [E, 4 * E], MDT, kind="ExternalInput")
    wout_d = nc.dram_tensor("wout", [4 * E, E], MDT, kind="ExternalInput")
    out_d = nc.dram_tensor("out", [L, E], F32, kind="ExternalOutput")

    if gates["bqk"]:
        g.bqk_d = nc.dram_tensor("bqk", [2 * E], F32, kind="ExternalInput")
    if gates["bv"]:
        g.bv_d = nc.dram_tensor("bv", [E], F32, kind="ExternalInput")
    if gates["bproj"]:
        g.bproj_d = nc.dram_tensor("bproj", [E], F32, kind="ExternalInput")
    if gates["bfc"]:
        g.bfce_d = nc.dram_tensor("bfce", [4 * E], F32, kind="ExternalInput")
        g.bfcl_d = nc.dram_tensor("bfcl", [4 * E], F32, kind="ExternalInput")
    if gates["bout"]:
        g.bout_d = nc.dram_tensor("bout", [E], F32, kind="ExternalInput")

    g.xv = x_d.rearrange("(t p) e -> p t e", p=P)            # [128, 8, 768]
    g.wqkv = wqk_d.rearrange("(c p) m -> p c m", p=P)        # [128, 6, 1536]
    g.wvv = wv_d.rearrange("(c p) m -> p c m", p=P)          # [128, 6, 768]
    g.wprojv = wproj_d.rearrange("(c p) m -> p c m", p=P)    # [128, 6, 768]
    g.wfcv = wfc_d.rearrange("(c p) m -> p c m", p=P)        # [128, 6, 3072]
    g.woutv = wout_d.rearrange("(c p) m -> p c m", p=P)      # [128, 24, 768]
    g.outv = out_d.rearrange("(t p) e -> p t e", p=P)

    with TileContext(nc) as tc, ExitStack() as es:
        g.pers = es.enter_context(tc.tile_pool(name="persist", bufs=1))
        _persist_setup(g, gates, MDT)

        g.bigp = es.enter_context(tc.tile_pool(name="big", bufs=1))
        g.x1p = es.enter_context(tc.tile_pool(name="x1p", bufs=LT))
        g.ln_scr = es.enter_context(tc.tile_pool(name="lnscr", bufs=2))
        g.wop = es.enter_context(tc.tile_pool(name="wop", bufs=1))

        g.z1T = g.bigp.tile([P, EC, L], MDT, tag="zT", name="z1T")
        g.OT = g.bigp.tile([P, EC, L], MDT, tag="ot", name="OT")
        g.wo_sb = g.wop.tile([P, KC2, E], MDT, name="wo")

        with (
            tc.tile_pool(name="xp", bufs=LT) as xp,
            tc.tile_pool(name="vaugp", bufs=1) as vaugp,
        ):
            g.xp = xp
            g.v_aug = vaugp.tile([P, LT, H, DA], MDT)

            _phB(g, gates, MDT, tc)
            _phC(g, gates, MDT, tc)

            g.z2T = g.bigp.tile([P, EC, L], MDT, tag="zT", name="z2T")
            _phD(g, gates, MDT, tc)

        with tc.tile_pool(name="htp", bufs=1) as htp:
            g.hT = htp.tile([P, KC2, L], MDT, name="hT")
            _phE(g, gates, MDT, tc)
            _phF(g, gates, MDT, tc)

    nc.finalize()
    return nc


def kernel(**inputs):
    global _last_results

    mm_dt_name = os.environ.get("KERNEL_MM_DT", "bf16")

    def arr(name):
        return np.ascontiguousarray(np.asarray(inputs[name], dtype=np.float32))

    x = arr("x")                       # [8, 1024, 768]
    g1 = arr("ln1_scale")
    b1 = arr("ln1_bias")
    w_qkv = arr("w_qkv")               # [768, 2304]
    b_qkv = arr("b_qkv")
    w_proj = arr("w_proj")
    b_proj = arr("b_proj")
    g2 = arr("ln2_scale")
    b2 = arr("ln2_bias")
    w_fc = arr("w_fc")
    b_fc = arr("b_fc")
    w_out = arr("w_out")
    b_out = arr("b_out")

    qscale = np.float32(1.0 / np.sqrt(D))

    w3 = w_qkv.reshape(E, H, 3, D)
    qw = (w3[:, :, 0, :].reshape(E, E) * qscale)
    kw = w3[:, :, 1, :].reshape(E, E)
    vw = w3[:, :, 2, :].reshape(E, E)
    wqk = np.ascontiguousarray(
        np.concatenate([qw, kw], axis=1) * g1[:, None]).astype(np.float32)
    wv = np.ascontiguousarray(vw * g1[:, None]).astype(np.float32)

    bq3 = (b1 @ w_qkv + b_qkv).reshape(H, 3, D)
    bqk = np.concatenate(
        [bq3[:, 0, :].reshape(E) * qscale, bq3[:, 1, :].reshape(E)]).astype(np.float32)
    bv = np.ascontiguousarray(bq3[:, 2, :].reshape(E)).astype(np.float32)

    wfc_p = np.ascontiguousarray(
        w_fc * g2[:, None] * np.float32(SELU_LAMBDA)).astype(np.float32)
    bfc_eff = (b2 @ w_fc + b_fc).astype(np.float32)
    bfce = (bfc_eff + np.float32(np.log(SELU_LA))).astype(np.float32)
    bfcl = (bfc_eff * np.float32(SELU_LAMBDA)).astype(np.float32)

    gates = {
        "bqk": bool(np.any(bqk != 0)),
        "bv": bool(np.any(bv != 0)),
        "bproj": bool(np.any(b_proj != 0)),
        "bfc": bool(np.any(bfc_eff != 0)),
        "bout": bool(np.any(b_out != 0)),
    }

    key = (tuple(sorted(gates.items())), mm_dt_name)
    if key not in _build_cache:
        _build_cache[key] = _build(gates, mm_dt_name)
    nc = _build_cache[key]

    wdt = np.float32 if mm_dt_name == "f32r" else ml_dtypes.bfloat16

    def wcast(a):
        return np.ascontiguousarray(a.astype(wdt))

    base = {
        "wqk": wcast(wqk), "wv": wcast(wv),
        "wproj": wcast(w_proj),
        "wfc": wcast(wfc_p),
        "wout": wcast(w_out),
    }
    if gates["bqk"]:
        base["bqk"] = bqk
    if gates["bv"]:
        base["bv"] = bv
    if gates["bproj"]:
        base["bproj"] = np.ascontiguousarray(b_proj)
    if gates["bfc"]:
        base["bfce"] = bfce
        base["bfcl"] = bfcl
    if gates["bout"]:
        base["bout"] = np.ascontiguousarray(b_out)

    in_maps = [dict(base, x=np.ascontiguousarray(x[c])) for c in range(NCORES)]
    res = bass_utils.run_bass_kernel_spmd(nc, in_maps, core_ids=list(range(NCORES)))
    _last_results = res
    out = np.stack([res.results[c]["out"] for c in range(NCORES)], axis=0)
    return out.astype(np.float32)


# revision 18
# speedup vs baseline: 1.8413x; 1.0117x over previous
"""Trainium2 Bass kernel for nn_AttentionBlock_68624987455817.

Pre-LN causal self-attention block + MLP (B=8, L=1024, E=768, H=12, D=64).

Sharding: data-parallel over batch B=8 across the 8 NeuronCores (one batch
element per core, weights replicated, no collectives). Each core runs the
full block on its [1024, 768] slice.

Key structure (v4):
- Fully pipelined per-tile LayerNorms: stats finalized per token tile (no
  all-tile barrier), LN apply + PE-array transpose + consumer matmuls
  software-pipelined in the PE queue.
- x and x1 SBUF-resident end to end (no DRAM scratch roundtrips).
- DMA triggers dispatch in program order from the sync queue, so all weight
  DMAs are emitted as early as their SBUF reservation window allows; wqk is
  resident (two large DMAs with 1.5KB descriptors instead of 24 tile DMAs
  with 256B descriptors).
- Attention: per head pair, qk of pair c+1 is emitted ahead of attention of
  pair c so the PE never waits on the softmax exp chain; P@V psum is copied
  to SBUF immediately (frees the bank) before the row-sum normalize chain;
  OT and hT are split into query halves to cut tile-granularity waits.
- proj: residual add fused with the LN2 row-sum via tensor_tensor_reduce.

Per-core dataflow (activations kept feature-major through the matmuls):
  phB    LN1 per x tile -> z tile -> PE transpose -> z1T [E, L]; v = z1 @ wv
         one tile behind (ones column per head makes P@V emit row-sums)
  phC    per head pair: qk chunks (q pre-scaled 1/sqrt(D)), then
         S^T = k_h^T q_h -> exp -> P^T (masked); [O^T; sums] = Vaug^T P^T
  phD    x1 = O @ wproj + x; per-tile LN2 -> z2 -> PE transpose -> z2T
  phE    hT = selu(wfc^T @ z2T)      (wfc pre-scaled by selu lambda)
  phF    out = h @ wout + x1         (token-major, two column passes)

Matmul operand dtype is selectable (KERNEL_MM_DT env): "bf16" (default;
rel err ~3e-3) or "f32r" (rel err ~2e-4; may not fit SBUF in this layout).
Accumulation is always fp32. Softmax skips the max-subtraction (|S| <= ~8
for LN'd inputs so exp cannot overflow in fp32).

LN scales fold into the following weight matrices host-side; LN biases and
all linear biases fold into per-feature biases that are only materialized
on-chip when nonzero (all zero for this problem's inputs).
"""
import os
import sys
from contextlib import ExitStack

sys.path.insert(0, "/opt/trn_rl_repo")

import numpy as np
import ml_dtypes

import concourse.bass as bass
from concourse import bacc
import concourse.mybir as mybir
from concourse.tile import TileContext
from concourse import bass_utils
from concourse.masks import make_identity

F32 = mybir.dt.float32
F32R = mybir.dt.float32r
BF16 = mybir.dt.bfloat16
AF = mybir.ActivationFunctionType
OP = mybir.AluOpType
AX = mybir.AxisListType

P = 128
L = 1024
E = 768
H = 12
D = 64
DA = D + 1           # V columns + ones column (row-sum trick)
EC = E // P          # 6 feature chunks
LT = L // P          # 8 token tiles
QC = L // 512        # 2 query chunks
KC2 = 4 * E // P     # 24 chunks of the MLP hidden dim
NCORES = 8

SELU_LAMBDA = 1.0507009873554805
SELU_ALPHA = 1.6732632423543772
SELU_LA = SELU_LAMBDA * SELU_ALPHA
LN_EPS = 1e-6

_last_results = None
_build_cache = {}


class _Ctx:
    pass


def _persist_setup(g, gates, MDT):
    nc, pers = g.nc, g.pers
    # mask_tri[p, f] = 1.0 if f >= p else 0.0 (keep where k <= q).
    # Built in f32 (f32r memset/affine_select fail walrus codegen).
    mask_f32 = pers.tile([P, P], F32)
    nc.gpsimd.memset(mask_f32[:], 0.0)
    nc.gpsimd.affine_select(
        out=mask_f32[:], in_=mask_f32[:],
        compare_op=OP.is_ge, fill=1.0, base=-1,
        pattern=[[-1, P]], channel_multiplier=1,
    )
    if MDT == F32R:
        g.mask_tri = mask_f32[:].bitcast(F32R)
    else:
        mask_b = pers.tile([P, P], BF16)
        nc.vector.tensor_copy(mask_b[:], mask_f32[:])
        g.mask_tri = mask_b[:]
    g.ones_f32 = pers.tile([P, LT * H], F32)
    nc.vector.memset(g.ones_f32[:], 1.0)
    g.eps_b = pers.tile([P, 1], F32)
    nc.vector.memset(g.eps_b[:], LN_EPS)
    g.lnla_b = pers.tile([P, 1], F32)
    nc.vector.memset(g.lnla_b[:], float(np.log(SELU_LA)))

    g.m1 = pers.tile([P, LT], F32)
    g.sq1 = pers.tile([P, LT], F32)
    g.r1 = pers.tile([P, LT], F32)
    g.tmp8 = pers.tile([P, LT], F32)
    g.m2 = pers.tile([P, LT], F32)
    g.sq2 = pers.tile([P, LT], F32)
    g.r2 = pers.tile([P, LT], F32)

    g.bqk_sb = g.bv_sb = g.bproj_sb = g.bfce_sb = g.bfcl_sb = g.bout_sb = None
    if gates["bqk"]:
        g.bqk_sb = pers.tile([P, 2 * EC], F32)
        nc.sync.dma_start(g.bqk_sb[:], g.bqk_d.rearrange("(c p) -> p c", p=P))
    if gates["bv"]:
        g.bv_sb = pers.tile([P, E], F32)
        nc.sync.dma_start(g.bv_sb[:], g.bv_d.to_broadcast((P, E)))
    if gates["bproj"]:
        g.bproj_sb = pers.tile([P, E], F32)
        nc.sync.dma_start(g.bproj_sb[:], g.bproj_d.to_broadcast((P, E)))
    if gates["bfc"]:
        g.bfce_sb = pers.tile([P, KC2], F32)
        nc.sync.dma_start(g.bfce_sb[:], g.bfce_d.rearrange("(c p) -> p c", p=P))
        g.bfcl_sb = pers.tile([P, KC2], F32)
        nc.sync.dma_start(g.bfcl_sb[:], g.bfcl_d.rearrange("(c p) -> p c", p=P))
    if gates["bout"]:
        g.bout_sb = pers.tile([P, E], F32)
        nc.sync.dma_start(g.bout_sb[:], g.bout_d.to_broadcast((P, E)))

    ident = pers.tile([P, P], F32)
    make_identity(nc, ident)
    g.ident_m = pers.tile([P, P], MDT)
    nc.vector.tensor_copy(g.ident_m[:], ident[:])


def _ln_finalize(g, msum, sqv, rv, t):
    """msum[:, t] holds the row SUM; sqv[:, t] the row sum of squares.
    Produces mean in msum[:, t] and 1/std in rv[:, t]."""
    nc = g.nc
    nc.vector.tensor_scalar_mul(msum[:, t:t + 1], msum[:, t:t + 1], 1.0 / E)
    nc.vector.tensor_tensor(g.tmp8[:, t:t + 1], msum[:, t:t + 1],
                            msum[:, t:t + 1], OP.mult)
    nc.vector.tensor_scalar_mul(sqv[:, t:t + 1], sqv[:, t:t + 1], 1.0 / E)
    nc.vector.tensor_tensor(sqv[:, t:t + 1], sqv[:, t:t + 1],
                            g.tmp8[:, t:t + 1], OP.subtract)
    nc.scalar.activation(sqv[:, t:t + 1], sqv[:, t:t + 1], AF.Sqrt,
                         bias=g.eps_b[:])
    nc.vector.reciprocal(rv[:, t:t + 1], sqv[:, t:t + 1])


def _phB(g, gates, MDT, tc):
    """LN1 -> z1T (PE transposes) with v = z1 @ wv pipelined one tile behind."""
    nc = g.nc
    with (
        tc.tile_pool(name="zp", bufs=3) as zp,
        tc.tile_pool(name="wvp", bufs=1) as wvp,
        tc.tile_pool(name="pstr", bufs=4, space="PSUM") as pstr,
        tc.tile_pool(name="psv", bufs=2, space="PSUM") as psv,
    ):
        g.wv_sb = wvp.tile([P, EC, E], MDT, name="wv")
        nc.sync.dma_start(g.wv_sb[:], g.wvv[:])
        nc.vector.tensor_copy(
            g.v_aug[:, :, :, D:DA],
            g.ones_f32[:].rearrange("p (t h o) -> p t h o", h=H, o=1))

        def make_v(t):
            def emit():
                for (c0, cw) in ((0, 512), (512, 256)):
                    pv = psv.tile([P, 512], F32, tag="vmm")
                    for kc in range(EC):
                        nc.tensor.matmul(
                            pv[:, :cw], g.z1T[:, kc, t * P:(t + 1) * P],
                            g.wv_sb[:, kc, c0:c0 + cw],
                            start=(kc == 0), stop=(kc == EC - 1),
                        )
                    h0 = c0 // D
                    nh = cw // D
                    dst = g.v_aug[:, t, h0:h0 + nh, 0:D]
                    if gates["bv"]:
                        nc.vector.tensor_tensor(
                            dst, pv[:, :cw].rearrange("p (h d) -> p h d", d=D),
                            g.bv_sb[:, c0:c0 + cw].rearrange(
                                "p (h d) -> p h d", d=D),
                            OP.add)
                    else:
                        nc.any.tensor_copy(
                            out=dst,
                            in_=pv[:, :cw].rearrange("p (h d) -> p h d", d=D))
            return emit

        v_emit = None
        for t in range(LT):
            xt = g.xtiles[t]
            nc.vector.tensor_reduce(g.m1[:, t:t + 1], xt[:], AX.X, OP.add)
            sqs = g.ln_scr.tile([P, E], F32, tag="sq")
            nc.scalar.activation(sqs[:], xt[:], AF.Square,
                                 accum_out=g.sq1[:, t:t + 1])
            _ln_finalize(g, g.m1, g.sq1, g.r1, t)
            zt = zp.tile([P, E], MDT, tag="z")
            nc.vector.tensor_scalar(
                zt[:], xt[:], g.m1[:, t:t + 1], g.r1[:, t:t + 1],
                OP.subtract, OP.mult,
            )
            for c in range(EC):
                pt = pstr.tile([P, P], MDT, tag="tr")
                nc.tensor.transpose(pt[:], zt[:, c * P:(c + 1) * P],
                                    g.ident_m[:])
                nc.any.tensor_copy(out=g.z1T[:, c, t * P:(t + 1) * P],
                                   in_=pt[:])
            if v_emit is not None:
                v_emit()
            v_emit = make_v(t)
        v_emit()


def _phC_qk(g, gates, c, qkpp, psqk):
    """qk matmuls for head pair c: oc=c (q), oc=EC+c (k)."""
    nc = g.nc
    qk_pair = qkpp.tile([P, 2, L], g.MDT, tag="qkpair")
    for i, oc in enumerate((c, EC + c)):
        psums = [psqk.tile([P, 512], F32, tag="mm", name=f"qkps{lc}")
                 for lc in range(QC)]
        for kc in range(EC):
            for lc in range(QC):
                nc.tensor.matmul(
                    psums[lc][:], g.wqk_sb[:, kc, oc * P:(oc + 1) * P],
                    g.z1T[:, kc, lc * 512:(lc + 1) * 512],
                    start=(kc == 0), stop=(kc == EC - 1),
                )
        for lc in range(QC):
            dst = qk_pair[:, i, lc * 512:(lc + 1) * 512]
            if gates["bqk"]:
                nc.scalar.activation(dst, psums[lc][:], AF.Identity,
                                     bias=g.bqk_sb[:, oc:oc + 1])
            else:
                nc.any.tensor_copy(out=dst, in_=psums[lc][:])
    return qk_pair


def _phC_attn(g, c, qc, qk_pair, PT, recp, rcbp, ps3s, ps3v):
    """S^T -> exp/mask -> P@V + normalize for head pair c, query chunk qc."""
    nc = g.nc
    q0 = qc * 512
    for kt in range(4 * qc, 4 * (qc + 1)):
        s0 = kt * P
        segs = [(s0, 512), (512, L)] if s0 < 512 else [(s0, L)]
        psss = []
        for par in range(2):
            rows = slice(par * D, par * D + D)
            pss = ps3s.tile([P, L], F32, tag="st", name=f"pss{par}")
            lhs = qk_pair[rows, 1, s0:s0 + P]
            for (a, b) in segs:
                nc.tensor.matmul(pss[:, a:b], lhs, qk_pair[rows, 0, a:b],
                                 start=True, stop=True)
            psss.append(pss)
        for par in range(2):
            pt_buf = PT[par]
            nc.scalar.activation(pt_buf[:, kt, s0:L], psss[par][:, s0:L],
                                 AF.Exp)
            nc.vector.tensor_tensor(
                pt_buf[:, kt, s0:s0 + P], pt_buf[:, kt, s0:s0 + P],
                g.mask_tri, OP.mult,
            )
    # P@V for both heads: lhsT = [V_h | 1] so psum row 64 carries the softmax
    # row-sums. The psum is copied to SBUF right away to free the bank; the
    # reciprocal (row 0) is broadcast into rows 1..64 of the same tile on the
    # otherwise-idle GpSimd (the custom DVE reciprocal reads garbage from
    # PSUM, hence the copy-first anyway).
    OTq = g.OTh[qc]
    for par in range(2):
        h = 2 * c + par
        pt_buf = PT[par]
        pso = ps3v.tile([P, 512], F32, tag="pv")
        kts = list(range(4 * (qc + 1)))
        for j, kt in enumerate(kts):
            a = max(kt * P, q0)
            nc.tensor.matmul(pso[0:DA, a - q0:512], g.v_aug[:, kt, h, :],
                             pt_buf[:, kt, a:q0 + 512],
                             start=(j == 0), stop=(j == len(kts) - 1))
        ocpy = recp.tile([P, 512], F32, tag="oc")
        nc.vector.tensor_copy(ocpy[0:D, :], pso[0:D, :])
        srow = rcbp.tile([P, 512], F32, tag="sr")
        nc.vector.tensor_copy(srow[0:1, :], pso[D:DA, :])
        rc = rcbp.tile([P, 512], F32, tag="rc")
        nc.vector.reciprocal_approx_fast(rc[0:1, :], srow[0:1, :])
        recb = rcbp.tile([P, 512], F32, tag="rb")
        nc.gpsimd.partition_broadcast(recb[0:D, :], rc[0:1, :])
        o_rows = slice(par * D, par * D + D)
        nc.vector.tensor_tensor(
            OTq[o_rows, c, :], ocpy[0:D, :], recb[0:D, :], OP.mult,
        )


def _phC(g, gates, MDT, tc):
    nc = g.nc
    with (
        tc.tile_pool(name="qkpp", bufs=2) as qkpp,
        tc.tile_pool(name="ptp", bufs=1) as ptp,
        tc.tile_pool(name="recp", bufs=2) as recp,
        tc.tile_pool(name="rcbp", bufs=1) as rcbp,
        tc.tile_pool(name="psqk", bufs=2, space="PSUM") as psqk,
        tc.tile_pool(name="ps3s", bufs=2, space="PSUM") as ps3s,
        tc.tile_pool(name="ps3v", bufs=2, space="PSUM") as ps3v,
    ):
        PT = [ptp.tile([P, LT, L], MDT, tag=f"pt{i}", name=f"pt{i}")
              for i in range(2)]
        # Prefetch the next pair's qk matmuls ahead of this pair's attention
        # in the PE queue: the qk matmuls have no ACT dependencies, so the PE
        # never stalls on the softmax exp chain.
        qk_next = _phC_qk(g, gates, 0, qkpp, psqk)
        for c in range(EC):  # head pair (2c, 2c+1)
            qk_pair = qk_next
            if c + 1 < EC:
                qk_next = _phC_qk(g, gates, c + 1, qkpp, psqk)
            for qc in range(QC):
                _phC_attn(g, c, qc, qk_pair, PT, recp, rcbp, ps3s, ps3v)


def _phD(g, gates, MDT, tc):
    """proj + residual (x1 SBUF-resident) + per-tile LN2 -> z2T.
    The residual add is fused with the LN2 row-sum via tensor_tensor_reduce."""
    nc = g.nc
    with (
        tc.tile_pool(name="z2p", bufs=3) as z2p,
        tc.tile_pool(name="psp4", bufs=3, space="PSUM") as ps4,
        tc.tile_pool(name="pstr2", bufs=4, space="PSUM") as pstr2,
    ):
        def make_tr2(t, z2t):
            def emit():
                for c in range(EC):
                    pt = pstr2.tile([P, P], MDT, tag="tr2")
                    nc.tensor.transpose(pt[:], z2t[:, c * P:(c + 1) * P],
                                        g.ident_m[:])
                    nc.any.tensor_copy(out=g.z2T[:, c, t * P:(t + 1) * P],
                                       in_=pt[:])
            return emit

        g.x1tiles = []
        tr_emit = None
        for t in range(LT):
            xt = g.xtiles[t]
            OTq = g.OTh[t // 4]
            tp = (t % 4) * P
            x1t = g.x1p.tile([P, E], F32, tag="x1")
            for pi, (c0, cw) in enumerate(((0, 512), (512, 256))):
                pt = ps4.tile([P, 512], F32, tag="mm")
                for kc in range(EC):
                    nc.tensor.matmul(
                        pt[:, :cw], OTq[:, kc, tp:tp + P],
                        g.wproj_sb[:, kc, c0:c0 + cw],
                        start=(kc == 0), stop=(kc == EC - 1),
                    )
                dst = x1t[:, c0:c0 + cw]
                if gates["bproj"]:
                    nc.vector.tensor_tensor(dst, pt[:, :cw],
                                            g.bproj_sb[:, c0:c0 + cw], OP.add)
                    nc.vector.tensor_tensor(dst, dst, xt[:, c0:c0 + cw],
                                            OP.add)
                    nc.vector.tensor_reduce(
                        g.tmp8[:, t:t + 1] if pi == 0 else g.m2[:, t:t + 1],
                        dst, AX.X, OP.add)
                    if pi == 1:
                        nc.vector.tensor_tensor(
                            g.m2[:, t:t + 1], g.m2[:, t:t + 1],
                            g.tmp8[:, t:t + 1], OP.add)
                else:
                    nc.vector.tensor_tensor(dst, pt[:, :cw],
                                            xt[:, c0:c0 + cw], OP.add)
            nc.vector.tensor_reduce(g.m2[:, t:t + 1], x1t[:], AX.X, OP.add)
            if False:
                pass
            g.x1tiles.append(x1t)
            sqs = g.ln_scr.tile([P, E], F32, tag="sq")
            nc.scalar.activation(sqs[:], x1t[:], AF.Square,
                                 accum_out=g.sq2[:, t:t + 1])
            _ln_finalize(g, g.m2, g.sq2, g.r2, t)
            z2t = z2p.tile([P, E], MDT, tag="z2")
            nc.vector.tensor_scalar(
                z2t[:], x1t[:], g.m2[:, t:t + 1], g.r2[:, t:t + 1],
                OP.subtract, OP.mult,
            )
            if tr_emit is not None:
                tr_emit()
            tr_emit = make_tr2(t, z2t)
        tr_emit()


def _phE(g, gates, MDT, tc, wfcp):
    """fc + selu -> hT halves (feature-major)."""
    nc = g.nc
    with (
        tc.tile_pool(name="selu", bufs=2) as slp,
        tc.tile_pool(name="ps5", bufs=4, space="PSUM") as ps5,
    ):
        for oc in range(KC2):
            ch, co = divmod(oc, 6)
            if oc == 6:
                # last wfc chunk: emitted once chunk 0's buffer is free
                wfc3 = wfcp.tile([P, EC, 6 * P], MDT, tag="wfc", name="wfc3")
                g.wfc_ch.append(wfc3)
                nc.sync.dma_start(wfc3[:], g.wfcv[:, :, 18 * P:24 * P])
            wt = g.wfc_ch[ch]
            for lc in range(QC):
                pt = ps5.tile([P, 512], F32, tag="mm")
                for kc in range(EC):
                    nc.tensor.matmul(
                        pt[:], wt[:, kc, co * P:(co + 1) * P],
                        g.z2T[:, kc, lc * 512:(lc + 1) * 512],
                        start=(kc == 0), stop=(kc == EC - 1),
                    )
                pe_t = slp.tile([P, 512], F32, tag="pe")
                bias = g.bfce_sb[:, oc:oc + 1] if gates["bfc"] else g.lnla_b[:]
                nc.scalar.activation(pe_t[:], pt[:], AF.Exp, bias=bias,
                                     scale=1.0 / SELU_LAMBDA)
                # in place: a = min(pe, LA) - LA
                nc.vector.tensor_scalar(pe_t[:], pe_t[:], SELU_LA, SELU_LA,
                                        OP.min, OP.subtract)
                dst = g.hTh[lc][:, oc, :]
                if gates["bfc"]:
                    rl = slp.tile([P, 512], F32, tag="rl")
                    nc.vector.tensor_scalar(rl[:], pt[:],
                                            g.bfcl_sb[:, oc:oc + 1],
                                            0.0, OP.add, OP.max)
                    nc.vector.tensor_tensor(dst, rl[:], pe_t[:], OP.add)
                else:
                    nc.vector.scalar_tensor_tensor(dst, pt[:], 0.0, pe_t[:],
                                                   OP.max, OP.add)


def _phF(g, gates, MDT, tc):
    """out = h @ wout + x1, two column passes, straight to DRAM."""
    nc = g.nc
    with (
        tc.tile_pool(name="osA", bufs=3) as osp,
        tc.tile_pool(name="ps6", bufs=4, space="PSUM") as ps6,
    ):
        for (c0, cw) in ((0, 512), (512, 256)):
            for t in range(LT):
                hTq = g.hTh[t // 4]
                tp = (t % 4) * P
                pt = ps6.tile([P, 512], F32, tag="mm")
                for kc in range(KC2):
                    nc.tensor.matmul(
                        pt[:, :cw], hTq[:, kc, tp:tp + P],
                        g.wo_sb[:, kc, c0:c0 + cw],
                        start=(kc == 0), stop=(kc == KC2 - 1),
                    )
                ot = osp.tile([P, 512], F32, tag="ot")
                if gates["bout"]:
                    nc.vector.tensor_tensor(ot[:, :cw], pt[:, :cw],
                                            g.bout_sb[:, c0:c0 + cw], OP.add)
                    nc.vector.tensor_tensor(ot[:, :cw], ot[:, :cw],
                                            g.x1tiles[t][:, c0:c0 + cw],
                                            OP.add)
                else:
                    nc.vector.tensor_tensor(ot[:, :cw], pt[:, :cw],
                                            g.x1tiles[t][:, c0:c0 + cw],
                                            OP.add)
                nc.sync.dma_start(g.outv[:, t, c0:c0 + cw], ot[:, :cw])


def _build(gates, mm_dt_name):
    MDT = {"f32r": F32R, "bf16": BF16}[mm_dt_name]

    nc = bacc.Bacc("TRN2", target_bir_lowering=False)
    g = _Ctx()
    g.nc = nc
    g.MDT = MDT

    x_d = nc.dram_tensor("x", [L, E], F32, kind="ExternalInput")
    wqk_d = nc.dram_tensor("wqk", [E, 2 * E], MDT, kind="ExternalInput")
    wv_d = nc.dram_tensor("wv", [E, E], MDT, kind="ExternalInput")
    wproj_d = nc.dram_tensor("wproj", [E, E], MDT, kind="ExternalInput")
    wfc_d = nc.dram_tensor("wfc", [E, 4 * E], MDT, kind="ExternalInput")
    wout_d = nc.dram_tensor("wout", [4 * E, E], MDT, kind="ExternalInput")
    out_d = nc.dram_tensor("out", [L, E], F32, kind="ExternalOutput")

    if gates["bqk"]:
        g.bqk_d = nc.dram_tensor("bqk", [2 * E], F32, kind="ExternalInput")
    if gates["bv"]:
        g.bv_d = nc.dram_tensor("bv", [E], F32, kind="ExternalInput")
    if gates["bproj"]:
        g.bproj_d = nc.dram_tensor("bproj", [E], F32, kind="ExternalInput")
    if gates["bfc"]:
        g.bfce_d = nc.dram_tensor("bfce", [4 * E], F32, kind="ExternalInput")
        g.bfcl_d = nc.dram_tensor("bfcl", [4 * E], F32, kind="ExternalInput")
    if gates["bout"]:
        g.bout_d = nc.dram_tensor("bout", [E], F32, kind="ExternalInput")

    g.xv = x_d.rearrange("(t p) e -> p t e", p=P)            # [128, 8, 768]
    g.wqkv = wqk_d.rearrange("(c p) m -> p c m", p=P)        # [128, 6, 1536]
    g.wvv = wv_d.rearrange("(c p) m -> p c m", p=P)          # [128, 6, 768]
    g.wprojv = wproj_d.rearrange("(c p) m -> p c m", p=P)    # [128, 6, 768]
    g.wfcv = wfc_d.rearrange("(c p) m -> p c m", p=P)        # [128, 6, 3072]
    g.woutv = wout_d.rearrange("(c p) m -> p c m", p=P)      # [128, 24, 768]
    g.outv = out_d.rearrange("(t p) e -> p t e", p=P)

    with TileContext(nc) as tc, ExitStack() as es:
        g.pers = es.enter_context(tc.tile_pool(name="persist", bufs=1))
        g.bigp = es.enter_context(tc.tile_pool(name="big", bufs=1))
        g.x1p = es.enter_context(tc.tile_pool(name="x1p", bufs=LT))
        g.xp = es.enter_context(tc.tile_pool(name="xp", bufs=LT))
        g.ln_scr = es.enter_context(tc.tile_pool(name="lnscr", bufs=2))
        g.wop = es.enter_context(tc.tile_pool(name="wop", bufs=1))

        # x tile DMAs first: the sync queue dispatches DMA triggers in
        # program order, and LN1 needs x before anything else.
        g.xtiles = []
        for t in range(LT):
            xt = g.xp.tile([P, E], F32, tag="x")
            nc.sync.dma_start(xt[:], g.xv[:, t, :])
            g.xtiles.append(xt)

        g.z1T = g.bigp.tile([P, EC, L], MDT, tag="zT", name="z1T")
        g.OTh = [g.bigp.tile([P, EC, 512], MDT, tag=f"ot{q}", name=f"OT{q}")
                 for q in range(QC)]
        g.wo_sb = g.wop.tile([P, KC2, E], MDT, name="wo")

        _persist_setup(g, gates, MDT)

        with (
            tc.tile_pool(name="vaugp", bufs=1) as vaugp,
            tc.tile_pool(name="wqkp", bufs=1) as wqkp,
        ):
            g.v_aug = vaugp.tile([P, LT, H, DA], MDT)
            g.wqk_sb = wqkp.tile([P, EC, 2 * E], MDT, name="wqk")
            nc.sync.dma_start(g.wqk_sb[:, :, 0:E], g.wqkv[:, :, 0:E])
            nc.sync.dma_start(g.wqk_sb[:, :, E:2 * E], g.wqkv[:, :, E:2 * E])

            _phB(g, gates, MDT, tc)
            _phC(g, gates, MDT, tc)

        with tc.tile_pool(name="wfcp", bufs=3) as wfcp:
            # weight prefetches for D/E/F: emitted here so their triggers
            # dispatch during phC/phD, not at first use.
            g.wproj_sb = g.bigp.tile([P, EC, E], MDT, tag="wp", name="wproj")
            nc.sync.dma_start(g.wproj_sb[:], g.wprojv[:])
            g.wfc_ch = []
            for ch in range(3):
                wt = wfcp.tile([P, EC, 6 * P], MDT, tag="wfc")
                nc.sync.dma_start(wt[:], g.wfcv[:, :, ch * 6 * P:(ch + 1) * 6 * P])
                g.wfc_ch.append(wt)
            for oq in range(4):
                nc.sync.dma_start(g.wo_sb[:, 6 * oq:6 * (oq + 1), :],
                                  g.woutv[:, 6 * oq:6 * (oq + 1), :])

            g.z2T = g.bigp.tile([P, EC, L], MDT, tag="zT", name="z2T")
            _phD(g, gates, MDT, tc)

            with tc.tile_pool(name="htp", bufs=1) as htp:
                g.hTh = [htp.tile([P, KC2, 512], MDT, tag=f"ht{q}",
                                  name=f"hT{q}") for q in range(QC)]
                _phE(g, gates, MDT, tc, wfcp)
                _phF(g, gates, MDT, tc)

    nc.finalize()
    return nc


def kernel(**inputs):
    global _last_results

    mm_dt_name = os.environ.get("KERNEL_MM_DT", "bf16")

    def arr(name):
        return np.ascontiguousarray(np.asarray(inputs[name], dtype=np.float32))

    x = arr("x")                       # [8, 1024, 768]
    g1 = arr("ln1_scale")
    b1 = arr("ln1_bias")
    w_qkv = arr("w_qkv")               # [768, 2304]
    b_qkv = arr("b_qkv")
    w_proj = arr("w_proj")
    b_proj = arr("b_proj")
    g2 = arr("ln2_scale")
    b2 = arr("ln2_bias")
    w_fc = arr("w_fc")
    b_fc = arr("b_fc")
    w_out = arr("w_out")
    b_out = arr("b_out")

    qscale = np.float32(1.0 / np.sqrt(D))

    w3 = w_qkv.reshape(E, H, 3, D)
    qw = (w3[:, :, 0, :].reshape(E, E) * qscale)
    kw = w3[:, :, 1, :].reshape(E, E)
    vw = w3[:, :, 2, :].reshape(E, E)
    wqk = np.ascontiguousarray(
        np.concatenate([qw, kw], axis=1) * g1[:, None]).astype(np.float32)
    wv = np.ascontiguousarray(vw * g1[:, None]).astype(np.float32)

    bq3 = (b1 @ w_qkv + b_qkv).reshape(H, 3, D)
    bqk = np.concatenate(
        [bq3[:, 0, :].reshape(E) * qscale, bq3[:, 1, :].reshape(E)]).astype(np.float32)
    bv = np.ascontiguousarray(bq3[:, 2, :].reshape(E)).astype(np.float32)

    wfc_p = np.ascontiguousarray(
        w_fc * g2[:, None] * np.float32(SELU_LAMBDA)).astype(np.float32)
    bfc_eff = (b2 @ w_fc + b_fc).astype(np.float32)
    bfce = (bfc_eff + np.float32(np.log(SELU_LA))).astype(np.float32)
    bfcl = (bfc_eff * np.float32(SELU_LAMBDA)).astype(np.float32)

    gates = {
        "bqk": bool(np.any(bqk != 0)),
        "bv": bool(np.any(bv != 0)),
        "bproj": bool(np.any(b_proj != 0)),
        "bfc": bool(np.any(bfc_eff != 0)),
        "bout": bool(np.any(b_out != 0)),
    }

    key = (tuple(sorted(gates.items())), mm_dt_name)
    if key not in _build_cache:
        _build_cache[key] = _build(gates, mm_dt_name)
    nc = _build_cache[key]

    wdt = np.float32 if mm_dt_name == "f32r" else ml_dtypes.bfloat16

    def wcast(a):
        return np.ascontiguousarray(a.astype(wdt))

    base = {
        "wqk": wcast(wqk), "wv": wcast(wv),
        "wproj": wcast(w_proj),
        "wfc": wcast(wfc_p),
        "wout": wcast(w_out),
    }
    if gates["bqk"]:
        base["bqk"] = bqk
    if gates["bv"]:
        base["bv"] = bv
    if gates["bproj"]:
        base["bproj"] = np.ascontiguousarray(b_proj)
    if gates["bfc"]:
        base["bfce"] = bfce
        base["bfcl"] = bfcl
    if gates["bout"]:
        base["bout"] = np.ascontiguousarray(b_out)

    in_maps = [dict(base, x=np.ascontiguousarray(x[c])) for c in range(NCORES)]
    res = bass_utils.run_bass_kernel_spmd(nc, in_maps, core_ids=list(range(NCORES)))
    _last_results = res
    out = np.stack([res.results[c]["out"] for c in range(NCORES)], axis=0)
    return out.astype(np.float32)
